# revision 5
# baseline (speedup 1.0000x reference)
"""CausalLocalAttention Trainium2 kernel (8-core SPMD, data-parallel, bf16).

Problem: B=4, S=4096, D=1024, H=16 heads, DH=64, window W=256 (block-local
causal attention), fp32 in/out.

Sharding: the 16384 tokens split into 8 contiguous 2048-token shards (block-
and batch-aligned), one per NeuronCore. Weights replicated. No collectives.

v3 design (vs v2, informed by HW microbenchmarks):
  * PE tile-mode switches (64-contract st vs 128-contract everything else)
    cost ~110ns each, and 64-contract matmuls stream at half rate UNLESS
    consecutive instructions alternate PE quadrants (tile_position row 0/64),
    which overlap and recover full rate. So attention matmuls are BATCHED:
    groups of 4 head-blocks emit all 8 st matmuls quadrant-interleaved, then
    (one group later) all 8 pv matmuls, with QKVO accumulation emits as
    spacers. Measured 340ns/head-block vs ~900ns for the v2 interleave.
  * the f32r ones-vector broadcast matmul is gone: V tiles carry 64 ones
    columns (width 128 = DH out + 64 ones), so the pv matmul itself lands
    the softmax denominator pre-broadcast in PSUM rows 64:128. Epilogue per
    head-block: ACT copy (rows 0:64 -> SBUF), DVE reciprocal (rows 64:128),
    Pool multiply -> at. No PE involvement, no extra tile mode.
  * PSUM: 2 acc banks (QKVO) + 4 st banks + 2 pv banks (each pv bank holds
    two head-blocks side by side) = 8 banks exactly.

Per-core layout (all SBUF activation tiles bf16):
  xt  [128, 8, CHUNK]    x^T feature-major (partition = feat%128, dim1 = feat//128)
  qt/kt [128, 8, CHUNK]  Q^T/K^T feature-major
  v3  [128, 8, 16, 128]  V token-major per i-tile, per head: DH cols + 64 ones
  at  [128, 8, CHUNK]    attn^T feature-major
  st  PSUM [128, 384]    S^T per (block, head): [k0 x q0:256 | k1 x q128:256]
  pv2 PSUM [128, 512]    two head-blocks: [outA|denA : 256] [outB|denB : 256]
"""
import sys
sys.path.insert(0, "/opt/trn_rl_repo")
import os
import numpy as np
from contextlib import ExitStack

import concourse.bass as bass
import concourse.tile as tile
from concourse import mybir
from concourse.bass_utils import run_bass_kernel_spmd
from concourse.vector_clock import ScopedClock

F32 = mybir.dt.float32
BF16 = mybir.dt.bfloat16

N_CORES = 8
B, S, D = 4, 4096, 1024
H, W, DH = 16, 256, 64
T_CORE = (B * S) // N_CORES      # 2048 tokens per core
CHUNK = 1024                     # tokens processed per chunk
N_CHUNK = T_CORE // CHUNK        # 2
NB = CHUNK // W                  # blocks per chunk (4)
NT = CHUNK // 128                # 128-token i-tiles per chunk (8)
SCALE = 1.0 / float(np.sqrt(DH))
NEG = -1e30
ST_W = W + 128                   # 384
VW = DH + 64                     # 128: v3 row width (out cols + ones cols)
GSZ = 4                          # head-blocks per attention group


# ---------------------------------------------------------------------------
# walrus on this toolchain allows only one sem wait per instruction; split
# extras onto same-engine NoOps inserted right before the instruction.
def _split_sync_waits(nc: bass.Bass, max_waits: int = 1) -> None:
    for fn in nc.m.functions:
        for bb in fn.blocks:
            insts = bb.instructions
            if not any(
                i.sync_info and i.sync_info.on_wait
                and len(i.sync_info.on_wait) > max_waits
                for i in insts
            ):
                continue
            new = []
            for inst in insts:
                si = inst.sync_info
                waits = list(si.on_wait) if (si and si.on_wait) else []
                if len(waits) > max_waits:
                    n_excess = len(waits) - max_waits
                    for w in waits[:n_excess]:
                        nop = mybir.InstNoOp(
                            name=f"WSPLIT-{nc.next_id()}", ins=[], outs=[]
                        )
                        nop.engine = inst.engine
                        nop.sync_info = mybir.SyncInfo(on_wait=[w], on_update=[])
                        nc.register_instruction(nop)
                        new.append(nop)
                    si.on_wait = waits[n_excess:]
                new.append(inst)
            bb.instructions = new


class _WTileContext(tile.TileContext):
    def _drain_and_barrier(self, tick_clock, wait_clock):
        drain_inst = self.nc.sync.drain()
        wait_clock.add_sem_waits(
            drain_inst.ins, ScopedClock({None: tick_clock.global_clock})
        )
        self.nc.all_engine_barrier()
        assert self.sems is not None
        popped = self.nc._tile_sem_poison_stack.pop()
        assert popped is self._sem_poison
        self.nc.clear_and_free_semaphores(list(self.sems.allocated().values()))
        self.nc.all_engine_barrier()

    def __exit__(self, exc_type, exc_val, exc_tb):
        ret = super().__exit__(exc_type, exc_val, exc_tb)
        if exc_type is None:
            _split_sync_waits(self.nc)
        return ret


# ---------------------------------------------------------------------------
def build_program(repeat: int = 1, use_loop: bool = False,
                  phases=("qk", "v", "attn", "o")) -> bass.Bass:
    """Build the SPMD one-core program (same for all cores)."""
    nc = bass.Bass("TRN2", target_bir_lowering=False, debug=False,
                   num_devices=N_CORES)

    xt_ap = nc.dram_tensor("xt", [N_CHUNK, 128, 8, CHUNK], BF16,
                           kind="ExternalInput").ap()
    wq_ap = nc.dram_tensor("wq", [128, 8, D], BF16, kind="ExternalInput").ap()
    wk_ap = nc.dram_tensor("wk", [128, 8, D], BF16, kind="ExternalInput").ap()
    wv_ap = nc.dram_tensor("wv", [128, 8, D], BF16, kind="ExternalInput").ap()
    wo_ap = nc.dram_tensor("wo", [128, 8, D], BF16, kind="ExternalInput").ap()
    bq_ap = nc.dram_tensor("bqr", [128, 8], F32, kind="ExternalInput").ap()
    bk_ap = nc.dram_tensor("bkr", [128, 8], F32, kind="ExternalInput").ap()
    bv_ap = nc.dram_tensor("bvb", [128, D], F32, kind="ExternalInput").ap()
    bo_ap = nc.dram_tensor("bob", [128, D], F32, kind="ExternalInput").ap()
    mf_ap = nc.dram_tensor("mfull", [128, ST_W], BF16, kind="ExternalInput").ap()
    y_ap = nc.dram_tensor("y", [T_CORE, D], F32, kind="ExternalOutput").ap()

    with _WTileContext(nc) as tc, ExitStack() as top:
        consts = top.enter_context(tc.tile_pool(name="consts", bufs=1))
        mf_sb = consts.tile([128, ST_W], BF16)
        bq_sb = consts.tile([128, 8], F32)
        bk_sb = consts.tile([128, 8], F32)
        bv_sb = consts.tile([128, D], F32)
        bo_sb = consts.tile([128, D], F32)
        nc.sync.dma_start(mf_sb[:], mf_ap[:])
        nc.sync.dma_start(bq_sb[:], bq_ap[:])
        nc.sync.dma_start(bk_sb[:], bk_ap[:])
        nc.sync.dma_start(bv_sb[:], bv_ap[:])
        nc.sync.dma_start(bo_sb[:], bo_ap[:])
        bv_h = bv_sb[:].rearrange("p (h d) -> p h d", h=H)

        # pools live for the whole program (NOT per-iteration): releasing a
        # pool acts as a coarse barrier that blocks next-iteration weight
        # prefetch behind the previous iteration's tail
        p_w = top.enter_context(tc.tile_pool(name="p_w", bufs=1))
        p_x = top.enter_context(tc.tile_pool(name="p_x", bufs=2))
        p_qkv = top.enter_context(tc.tile_pool(name="p_qkv", bufs=1))
        p_v3 = top.enter_context(tc.tile_pool(name="p_v3", bufs=1))
        p_at = top.enter_context(tc.tile_pool(name="p_at", bufs=1))
        p_pt = top.enter_context(tc.tile_pool(name="p_pt", bufs=8))
        p_pvs = top.enter_context(tc.tile_pool(name="p_pvs", bufs=4))
        p_y = top.enter_context(tc.tile_pool(name="p_y", bufs=2))
        ps_main = top.enter_context(
            tc.tile_pool(name="ps_main", bufs=2, space="PSUM"))
        ps_pv = top.enter_context(
            tc.tile_pool(name="ps_pv", bufs=2, space="PSUM"))

        # v3 allocated once: the 64 ones-columns are written a single time
        # (emit_v only ever writes cols 0:DH)
        v3_sb = p_v3.tile([128, NT, H, VW], BF16, name="v3s")
        nc.gpsimd.memset(v3_sb[:, :, :, DH:VW], 1.0)

        rep_iter = [None] if use_loop else list(range(repeat))
        loop_cm = tc.For_i(
            0, repeat, 1,
            hint_engines=(mybir.EngineType.PE, mybir.EngineType.DVE,
                          mybir.EngineType.Activation, mybir.EngineType.SP,
                          mybir.EngineType.Pool),
        ) if use_loop else None
        if loop_cm is not None:
            loop_cm.__enter__()
        for _rep in rep_iter:
            if True:
                wq_sb = p_w.tile([128, 8, D], BF16, tag="wq")
                wk_sb = p_w.tile([128, 8, D], BF16, tag="wk")
                wv_sb = p_w.tile([128, 8, D], BF16, tag="wv")
                wo_sb = p_w.tile([128, 8, D], BF16, tag="wo")
                xts = [p_x.tile([128, 8, CHUNK], BF16, tag="xt",
                                name=f"xt{i}") for i in range(N_CHUNK)]

                # --- input DMA stream (SP queue, consumption order) ---
                for k in range(8):
                    nc.sync.dma_start(xts[0][:, k, :], xt_ap[0, :, k, :])
                for w_sb, w_ap in ((wq_sb, wq_ap), (wk_sb, wk_ap)):
                    nc.sync.dma_start(w_sb[:, :, 0:512], w_ap[:, :, 0:512])
                    nc.sync.dma_start(w_sb[:, :, 512:D], w_ap[:, :, 512:D])
                nc.sync.dma_start(wv_sb[:], wv_ap[:])
                nc.sync.dma_start(wo_sb[:], wo_ap[:])
                if N_CHUNK > 1:
                    nc.sync.dma_start(xts[1][:], xt_ap[1])

                carry = []
                for c in range(N_CHUNK):
                    xt_sb = xts[c]
                    qt_sb = p_qkv.tile([128, 8, CHUNK], BF16, tag="qt")
                    kt_sb = p_qkv.tile([128, 8, CHUNK], BF16, tag="kt")

                    def emit_qk(w_sb, bias_sb, dst, m, n, xt_sb=xt_sb):
                        acc = ps_main.tile([128, 512], F32, tag="acc")
                        for k in range(8):
                            nc.tensor.matmul(
                                acc[:],
                                w_sb[:, k, m * 128:(m + 1) * 128],
                                xt_sb[:, k, n * 512:(n + 1) * 512],
                                start=(k == 0), stop=(k == 7),
                            )
                        nc.scalar.activation(
                            dst[:, m, n * 512:(n + 1) * 512],
                            acc[:],
                            mybir.ActivationFunctionType.Identity,
                            bias=bias_sb[:, m:m + 1],
                        )

                    def emit_qk_n1(m, pi):
                        if pi == 0:
                            emit_qk(wq_sb, bq_sb, qt_sb, m, 1)
                        else:
                            emit_qk(wk_sb, bk_sb, kt_sb, m, 1)

                    if "qk" not in phases:
                        nc.vector.memset(qt_sb[:], 0.5)
                        nc.vector.memset(kt_sb[:], 0.5)

                    # at_sb allocated after S1 so the previous chunk's carried
                    # O readers are registered before this buffer is reused;
                    # see below.

                    def emit_v(i, h2, xt_sb=xt_sb):
                        acc = ps_main.tile([128, 512], F32, tag="acc")
                        for k in range(8):
                            nc.tensor.matmul(
                                acc[:],
                                xt_sb[:, k, i * 128:(i + 1) * 128],
                                wv_sb[:, k, h2 * 512:(h2 + 1) * 512],
                                start=(k == 0), stop=(k == 7),
                            )
                        nc.vector.tensor_add(
                            v3_sb[:, i, h2 * 8:(h2 + 1) * 8, 0:DH],
                            acc[:].rearrange("p (h d) -> p h d", h=8),
                            bv_h[:, h2 * 8:(h2 + 1) * 8, :],
                        )

                    y_tiles = {}

                    def emit_o(i, h2, y_tiles=y_tiles, c=c):
                        # at_sb is bound late (after S1) via at_ref
                        at_sb = at_ref[0]
                        if h2 == 0:
                            y_tiles[i] = p_y.tile([128, D], F32, tag="y",
                                                  name=f"yt{i}")
                        y_t = y_tiles[i]
                        acc = ps_main.tile([128, 512], F32, tag="acc")
                        for k in range(8):
                            nc.tensor.matmul(
                                acc[:],
                                at_sb[:, k, i * 128:(i + 1) * 128],
                                wo_sb[:, k, h2 * 512:(h2 + 1) * 512],
                                start=(k == 0), stop=(k == 7),
                            )
                        nc.vector.tensor_add(
                            y_t[:, h2 * 512:(h2 + 1) * 512], acc[:],
                            bo_sb[:, h2 * 512:(h2 + 1) * 512])
                        if h2 == 1:
                            nc.sync.dma_start(
                                y_ap[c * CHUNK + i * 128:
                                     c * CHUNK + (i + 1) * 128, :],
                                y_t[:])

                    # ---- attention group machinery ----
                    sts = {}     # (b, h) -> (st_psum, pt)
                    pts = {}

                    def emit_st_batch(group, qt_sb=qt_sb, kt_sb=kt_sb):
                        # group: list of (b, h); emit quadrant-interleaved
                        # pairs, then mask-add + exp per head-block
                        tiles = {}
                        for (b, h) in group:
                            st_t = ps_main.tile(
                                [128, ST_W], F32, tag="st", bufs=4,
                                name=f"st{b}_{h}")
                            tiles[(b, h)] = st_t
                        for pi in range(0, len(group), 2):
                            pair = group[pi:pi + 2]
                            for (b, h) in pair:
                                t0 = b * W
                                hp = (h % 2) * 64
                                j = h // 2
                                st = tiles[(b, h)]
                                nc.tensor.matmul(
                                    st[:, 0:W],
                                    kt_sb[hp:hp + 64, j, t0:t0 + 128],
                                    qt_sb[hp:hp + 64, j, t0:t0 + W],
                                    start=True, stop=True,
                                    tile_position=(hp, 0))
                            for (b, h) in pair:
                                t0 = b * W
                                hp = (h % 2) * 64
                                j = h // 2
                                st = tiles[(b, h)]
                                nc.tensor.matmul(
                                    st[:, W:ST_W],
                                    kt_sb[hp:hp + 64, j, t0 + 128:t0 + W],
                                    qt_sb[hp:hp + 64, j, t0 + 128:t0 + W],
                                    start=True, stop=True,
                                    tile_position=(hp, 0),
                                    skip_group_check=True)
                        for (b, h) in group:
                            st = tiles[(b, h)]
                            pt = p_pt.tile([128, ST_W], BF16, tag="pt")
                            nc.scalar.activation(
                                pt[:], st[:],
                                mybir.ActivationFunctionType.Exp, scale=SCALE)
                            nc.vector.tensor_mul(pt[:], pt[:], mf_sb[:])
                            pts[(b, h)] = pt

                    def emit_pv_batch(group, at_getter=lambda: at_ref[0]):
                        at_sb = at_getter()
                        for pi in range(0, len(group), 2):
                            pair = group[pi:pi + 2]
                            pv2 = ps_pv.tile([128, 2 * W], F32, tag="pv2")
                            for ci, (b, h) in enumerate(pair):
                                pt = pts.pop((b, h))
                                c0 = ci * W
                                nc.tensor.matmul(
                                    pv2[:, c0:c0 + W],
                                    v3_sb[:, 2 * b, h, :], pt[:, 0:W],
                                    start=True, stop=False,
                                    skip_group_check=True)
                                nc.tensor.matmul(
                                    pv2[:, c0 + 128:c0 + W],
                                    v3_sb[:, 2 * b + 1, h, :], pt[:, W:ST_W],
                                    start=False, stop=True,
                                    skip_group_check=True)
                            for ci, (b, h) in enumerate(pair):
                                c0 = ci * W
                                t0 = b * W
                                hp = (h % 2) * 64
                                j = h // 2
                                rec = p_pvs.tile([64, W], F32, tag="rec",
                                                 bufs=4)
                                nc.vector.reciprocal(
                                    rec[:], pv2[64:128, c0:c0 + W])
                                pvs = p_pvs.tile([64, W], F32, tag="pvs",
                                                 bufs=4)
                                nc.scalar.copy(pvs[:], pv2[0:64, c0:c0 + W])
                                nc.gpsimd.tensor_mul(
                                    at_sb[hp:hp + 64, j, t0:t0 + W],
                                    pvs[:], rec[:])

                    do_attn = "attn" in phases
                    do_v = "v" in phases
                    do_o = "o" in phases
                    do_qk = "qk" in phases

                    # S1: QK n=0 accs interleaved with carried O accs and the
                    # V accs for blocks 0-1.
                    s1_fill = list(carry)
                    carry = []
                    if do_v:
                        s1_fill += [(emit_v, i, h2)
                                    for i in (0, 1, 2, 3) for h2 in (0, 1)]
                    if do_qk:
                        for m in range(8):
                            emit_qk(wq_sb, bq_sb, qt_sb, m, 0)
                            if s1_fill:
                                fn, a1, a2 = s1_fill.pop(0)
                                fn(a1, a2)
                            emit_qk(wk_sb, bk_sb, kt_sb, m, 0)
                            if s1_fill:
                                fn, a1, a2 = s1_fill.pop(0)
                                fn(a1, a2)
                    for fn, a1, a2 in s1_fill:
                        fn(a1, a2)

                    # bind at AFTER carried O emits are registered
                    at_sb = p_at.tile([128, 8, CHUNK], BF16, tag="at")
                    at_ref = [at_sb]
                    if "v" not in phases:
                        nc.vector.memset(v3_sb[:, :, :, 0:DH], 0.01)
                    if "attn" not in phases:
                        nc.vector.memset(at_sb[:], 0.01)

                    # filler queue, deadline-ordered: early m-tiles of QK n=1
                    # and V i4/i5 first (blocks 2-3 need them from group 8 on)
                    fillq = []
                    if do_qk:
                        qk1 = [(emit_qk_n1, m, pi)
                               for m in range(8) for pi in (0, 1)]
                    else:
                        qk1 = []
                    if do_v:
                        vf = [(emit_v, i, h2)
                              for i in (4, 5, 6, 7) for h2 in (0, 1)]
                    else:
                        vf = []
                    # deadline order (slot p+1 consumes pos p; st(g) needs
                    # its m-tiles by pos 2g-2, pv(g) needs v3 by pos 2g):
                    # qk-n1 m0,m1 | v i4 | m2 | v i5 | m3..m7 | v i6,i7
                    fillq = (qk1[0:4] + vf[0:2] + qk1[4:6] + vf[2:4]
                             + qk1[6:16] + vf[4:8])

                    ready_o = []

                    def filler():
                        if fillq:
                            fn, a1, a2 = fillq.pop(0)
                            fn(a1, a2)
                        elif ready_o and do_o:
                            emit_o(*ready_o.pop(0))

                    if do_attn:
                        hbs = [(b, h) for b in range(NB) for h in range(H)]
                        groups = [hbs[i:i + GSZ]
                                  for i in range(0, len(hbs), GSZ)]
                        prev = None
                        for gi, group in enumerate(groups):
                            emit_st_batch(group)
                            filler()
                            if prev is not None:
                                emit_pv_batch(prev)
                                b_done = prev[-1][0]
                                if prev[-1][1] == H - 1 and do_o:
                                    ready_o.extend(
                                        [(2 * b_done, 0), (2 * b_done, 1),
                                         (2 * b_done + 1, 0),
                                         (2 * b_done + 1, 1)])
                            filler()
                            prev = group
                        emit_pv_batch(prev)
                        if prev[-1][1] == H - 1 and do_o:
                            b_done = prev[-1][0]
                            ready_o.extend(
                                [(2 * b_done, 0), (2 * b_done, 1),
                                 (2 * b_done + 1, 0), (2 * b_done + 1, 1)])
                        for fn, a1, a2 in fillq:
                            fn(a1, a2)
                        fillq = []
                        n_carry = 8 if (c + 1 < N_CHUNK and do_qk
                                        and do_o) else 0
                        if n_carry:
                            carry = [(emit_o, i, h2)
                                     for i, h2 in ready_o[-n_carry:]]
                            ready_o = ready_o[:-n_carry]
                        for i, h2 in ready_o:
                            emit_o(i, h2)
                    else:
                        for fn, a1, a2 in fillq:
                            fn(a1, a2)
                        if do_o:
                            for i in range(NT):
                                for h2 in (0, 1):
                                    emit_o(i, h2)
        if loop_cm is not None:
            loop_cm.__exit__(None, None, None)
    return nc


# ---------------------------------------------------------------------------
_CACHE: dict = {}


def _host_prep(x, Wq, bq, Wk, bk, Wv, bv, Wo, bo):
    import ml_dtypes
    BF = ml_dtypes.bfloat16
    x = np.asarray(x, np.float32)
    x_flat = np.ascontiguousarray(x.reshape(B * S, D)).astype(BF)
    mf = np.ones((128, ST_W), np.float32)
    for p in range(128):
        mf[p, :p] = 0.0
        mf[p, W:W + p] = 0.0
    mf = mf.astype(BF)

    def wfmt(Wm):
        # [128, 8, D]: wfmt[p, k, c] = W[k*128 + p, c]
        return np.ascontiguousarray(
            np.asarray(Wm, np.float32).reshape(8, 128, D)
            .transpose(1, 0, 2).astype(BF))

    def xfmt(shard_x):
        # shard_x [T_CORE, D] -> [N_CHUNK, 128, 8, CHUNK]
        xt = shard_x.T  # [D, T_CORE]
        return np.ascontiguousarray(
            xt.reshape(8, 128, N_CHUNK, CHUNK).transpose(2, 1, 0, 3))

    shard = {
        "xt": np.stack([
            xfmt(x_flat[cix * T_CORE:(cix + 1) * T_CORE])
            for cix in range(N_CORES)
        ]),
    }
    repl = {
        "wq": wfmt(Wq),
        "wk": wfmt(Wk),
        "wv": wfmt(Wv),
        "wo": wfmt(Wo),
        "bqr": np.ascontiguousarray(np.asarray(bq, np.float32).reshape(8, 128).T),
        "bkr": np.ascontiguousarray(np.asarray(bk, np.float32).reshape(8, 128).T),
        "bvb": np.ascontiguousarray(np.tile(np.asarray(bv, np.float32), (128, 1))),
        "bob": np.ascontiguousarray(np.tile(np.asarray(bo, np.float32), (128, 1))),
        "mfull": mf,
    }
    return shard, repl


def _make_runner(repeat: int, use_loop: bool = False,
                 phases=("qk", "v", "attn", "o")):
    """Build program + cached jitted executable. Returns (run, n_outs info)."""
    import jax
    from jax.sharding import Mesh, PartitionSpec
    from jax.experimental.shard_map import shard_map
    from concourse import bass2jax
    from concourse.bass2jax import _bass_exec_p, install_neuronx_cc_hook

    install_neuronx_cc_hook()
    nc = build_program(repeat, use_loop, phases)
    partition_name = (
        nc.partition_id_tensor.name if nc.partition_id_tensor else None
    )
    in_names, out_names, out_avals = [], [], []
    import jax.core
    for alloc in nc.m.functions[0].allocations:
        if not isinstance(alloc, mybir.MemoryLocationSet):
            continue
        name = alloc.memorylocations[0].name
        if alloc.kind == "ExternalInput":
            if name != partition_name:
                in_names.append(name)
        elif alloc.kind == "ExternalOutput":
            out_names.append(name)
            out_avals.append(jax.core.ShapedArray(
                tuple(alloc.tensor_shape), mybir.dt.np(alloc.dtype)))
    all_in_names = list(in_names) + list(out_names)
    if partition_name is not None:
        all_in_names.append(partition_name)

    def _body(*args):
        operands = list(args)
        if partition_name is not None:
            operands.append(bass2jax.partition_id_tensor())
        return tuple(_bass_exec_p.bind(
            *operands,
            out_avals=tuple(out_avals),
            in_names=tuple(all_in_names),
            out_names=tuple(out_names),
            lowering_input_output_aliases=(),
            sim_require_finite=True,
            sim_require_nnan=True,
            nc=nc,
        ))

    import jax as _jax
    devices = _jax.devices()[:N_CORES]
    mesh = Mesh(np.asarray(devices), ("core",))
    SHARDED_INS = {"xt"}
    in_specs = tuple(
        PartitionSpec("core") if n in SHARDED_INS else PartitionSpec()
        for n in in_names
    ) + (PartitionSpec("core"),) * len(out_names)
    out_specs = (PartitionSpec("core"),) * len(out_names)
    sharded = _jax.jit(
        shard_map(_body, mesh=mesh, in_specs=in_specs,
                  out_specs=out_specs, check_rep=False),
        keep_unused=True,
    )

    from jax.sharding import NamedSharding
    sh_core = NamedSharding(mesh, PartitionSpec("core"))
    sh_repl = NamedSharding(mesh, PartitionSpec())

    def _args(shard_arrs: dict, repl_arrs: dict):
        args, shs = [], []
        for n in in_names:
            if n in SHARDED_INS:
                a = shard_arrs[n]
                args.append(a.reshape(a.shape[0] * a.shape[1], *a.shape[2:]))
                shs.append(sh_core)
            else:
                args.append(repl_arrs[n])
                shs.append(sh_repl)
        for av in out_avals:
            args.append(np.zeros((N_CORES * av.shape[0], *av.shape[1:]),
                                 av.dtype))
            shs.append(sh_core)
        return args, shs

    class Runner:
        def stage(self, shard_arrs, repl_arrs):
            args, shs = _args(shard_arrs, repl_arrs)
            dargs = [_jax.device_put(a, s) for a, s in zip(args, shs)]
            _jax.block_until_ready(dargs)
            return dargs

        def exec_staged(self, dargs):
            outs = sharded(*dargs)
            _jax.block_until_ready(outs)
            return outs

        def run(self, shard_arrs, repl_arrs):
            args, _ = _args(shard_arrs, repl_arrs)
            outs = sharded(*args)
            _jax.block_until_ready(outs)
            return {
                name: np.asarray(outs[i]).reshape(N_CORES, *out_avals[i].shape)
                for i, name in enumerate(out_names)
            }

    return Runner()


def get_runner(repeat: int = 1, use_loop: bool = False,
               phases=("qk", "v", "attn", "o")):
    key = ("runner", repeat, use_loop, tuple(phases))
    if key not in _CACHE:
        _CACHE[key] = _make_runner(repeat, use_loop, phases)
    return _CACHE[key]


def kernel(**inputs) -> np.ndarray:
    runner = get_runner(repeat=1)
    shard, repl = _host_prep(**inputs)
    out = runner.run(shard, repl)
    y = out["y"].reshape(B * S, D)
    return y.reshape(B, S, D).astype(np.float32)


# revision 6
# speedup vs baseline: 1.5378x; 1.5378x over previous
"""CausalLocalAttention Trainium2 kernel (8-core SPMD, data-parallel, bf16).

Problem: B=4, S=4096, D=1024, H=16 heads, DH=64, window W=256 (block-local
causal attention), fp32 in/out.

Sharding: the 16384 tokens split into 8 contiguous 2048-token shards (block-
and batch-aligned), one per NeuronCore. Weights replicated. No collectives.

v3 design (vs v2, informed by HW microbenchmarks):
  * PE tile-mode switches (64-contract st vs 128-contract everything else)
    cost ~110ns each, and 64-contract matmuls stream at half rate UNLESS
    consecutive instructions alternate PE quadrants (tile_position row 0/64),
    which overlap and recover full rate. So attention matmuls are BATCHED:
    groups of 4 head-blocks emit all 8 st matmuls quadrant-interleaved, then
    (one group later) all 8 pv matmuls, with QKVO accumulation emits as
    spacers. Measured 340ns/head-block vs ~900ns for the v2 interleave.
  * the f32r ones-vector broadcast matmul is gone: V tiles carry 64 ones
    columns (width 128 = DH out + 64 ones), so the pv matmul itself lands
    the softmax denominator pre-broadcast in PSUM rows 64:128. Epilogue per
    head-block: ACT copy (rows 0:64 -> SBUF), DVE reciprocal (rows 64:128),
    Pool multiply -> at. No PE involvement, no extra tile mode.
  * PSUM: 2 acc banks (QKVO) + 4 st banks + 2 pv banks (each pv bank holds
    two head-blocks side by side) = 8 banks exactly.

Per-core layout (all SBUF activation tiles bf16):
  xt  [128, 8, CHUNK]    x^T feature-major (partition = feat%128, dim1 = feat//128)
  qt/kt [128, 8, CHUNK]  Q^T/K^T feature-major
  v3  [128, 8, 16, 128]  V token-major per i-tile, per head: DH cols + 64 ones
  at  [128, 8, CHUNK]    attn^T feature-major
  st  PSUM [128, 384]    S^T per (block, head): [k0 x q0:256 | k1 x q128:256]
  pv2 PSUM [128, 512]    two head-blocks: [outA|denA : 256] [outB|denB : 256]
"""
import sys
sys.path.insert(0, "/opt/trn_rl_repo")
import os
import numpy as np
from contextlib import ExitStack

import concourse.bass as bass
import concourse.tile as tile
from concourse import mybir
from concourse.bass_utils import run_bass_kernel_spmd
from concourse.vector_clock import ScopedClock

F32 = mybir.dt.float32
BF16 = mybir.dt.bfloat16

N_CORES = 8
B, S, D = 4, 4096, 1024
H, W, DH = 16, 256, 64
T_CORE = (B * S) // N_CORES      # 2048 tokens per core
CHUNK = 1024                     # tokens processed per chunk
N_CHUNK = T_CORE // CHUNK        # 2
NB = CHUNK // W                  # blocks per chunk (4)
NT = CHUNK // 128                # 128-token i-tiles per chunk (8)
SCALE = 1.0 / float(np.sqrt(DH))
NEG = -1e30
ST_W = W + 128                   # 384
VW = DH + 64                     # 128: v3 row width (out cols + ones cols)
GSZ = 4                          # head-blocks per attention group


# ---------------------------------------------------------------------------
# walrus on this toolchain allows only one sem wait per instruction; split
# extras onto same-engine NoOps inserted right before the instruction.
def _split_sync_waits(nc: bass.Bass, max_waits: int = 1) -> None:
    for fn in nc.m.functions:
        for bb in fn.blocks:
            insts = bb.instructions
            if not any(
                i.sync_info and i.sync_info.on_wait
                and len(i.sync_info.on_wait) > max_waits
                for i in insts
            ):
                continue
            new = []
            for inst in insts:
                si = inst.sync_info
                waits = list(si.on_wait) if (si and si.on_wait) else []
                if len(waits) > max_waits:
                    n_excess = len(waits) - max_waits
                    for w in waits[:n_excess]:
                        nop = mybir.InstNoOp(
                            name=f"WSPLIT-{nc.next_id()}", ins=[], outs=[]
                        )
                        nop.engine = inst.engine
                        nop.sync_info = mybir.SyncInfo(on_wait=[w], on_update=[])
                        nc.register_instruction(nop)
                        new.append(nop)
                    si.on_wait = waits[n_excess:]
                new.append(inst)
            bb.instructions = new


class _WTileContext(tile.TileContext):
    def _drain_and_barrier(self, tick_clock, wait_clock):
        drain_inst = self.nc.sync.drain()
        wait_clock.add_sem_waits(
            drain_inst.ins, ScopedClock({None: tick_clock.global_clock})
        )
        self.nc.all_engine_barrier()
        assert self.sems is not None
        popped = self.nc._tile_sem_poison_stack.pop()
        assert popped is self._sem_poison
        self.nc.clear_and_free_semaphores(list(self.sems.allocated().values()))
        self.nc.all_engine_barrier()

    def __exit__(self, exc_type, exc_val, exc_tb):
        ret = super().__exit__(exc_type, exc_val, exc_tb)
        if exc_type is None:
            _split_sync_waits(self.nc)
        return ret


# ---------------------------------------------------------------------------
def build_program(repeat: int = 1, use_loop: bool = False,
                  phases=("qk", "v", "attn", "o")) -> bass.Bass:
    """Build the SPMD one-core program (same for all cores)."""
    nc = bass.Bass("TRN2", target_bir_lowering=False, debug=False,
                   num_devices=N_CORES)

    xt_ap = nc.dram_tensor("xt", [N_CHUNK, 128, 8, CHUNK], BF16,
                           kind="ExternalInput").ap()
    wq_ap = nc.dram_tensor("wq", [128, 8, D], BF16, kind="ExternalInput").ap()
    wk_ap = nc.dram_tensor("wk", [128, 8, D], BF16, kind="ExternalInput").ap()
    wv_ap = nc.dram_tensor("wv", [128, 8, D], BF16, kind="ExternalInput").ap()
    wo_ap = nc.dram_tensor("wo", [128, 8, D], BF16, kind="ExternalInput").ap()
    bq_ap = nc.dram_tensor("bqr", [128, 8], F32, kind="ExternalInput").ap()
    bk_ap = nc.dram_tensor("bkr", [128, 8], F32, kind="ExternalInput").ap()
    bv_ap = nc.dram_tensor("bvb", [128, D], F32, kind="ExternalInput").ap()
    bo_ap = nc.dram_tensor("bob", [128, D], F32, kind="ExternalInput").ap()
    mf_ap = nc.dram_tensor("mfull", [128, ST_W], BF16, kind="ExternalInput").ap()
    y_ap = nc.dram_tensor("y", [T_CORE, D], F32, kind="ExternalOutput").ap()

    with _WTileContext(nc) as tc, ExitStack() as top:
        consts = top.enter_context(tc.tile_pool(name="consts", bufs=1))
        mf_sb = consts.tile([128, ST_W], BF16)
        bq_sb = consts.tile([128, 8], F32)
        bk_sb = consts.tile([128, 8], F32)
        bv_sb = consts.tile([128, D], F32)
        bo_sb = consts.tile([128, D], F32)
        nc.sync.dma_start(mf_sb[:], mf_ap[:])
        nc.sync.dma_start(bq_sb[:], bq_ap[:])
        nc.sync.dma_start(bk_sb[:], bk_ap[:])
        nc.sync.dma_start(bv_sb[:], bv_ap[:])
        nc.sync.dma_start(bo_sb[:], bo_ap[:])
        bv_h = bv_sb[:].rearrange("p (h d) -> p h d", h=H)

        # pools live for the whole program (NOT per-iteration): releasing a
        # pool acts as a coarse barrier that blocks next-iteration weight
        # prefetch behind the previous iteration's tail
        p_w = top.enter_context(tc.tile_pool(name="p_w", bufs=1))
        p_x = top.enter_context(tc.tile_pool(name="p_x", bufs=2))
        p_qkv = top.enter_context(tc.tile_pool(name="p_qkv", bufs=1))
        p_v3 = top.enter_context(tc.tile_pool(name="p_v3", bufs=1))
        p_at = top.enter_context(tc.tile_pool(name="p_at", bufs=1))
        p_pt = top.enter_context(tc.tile_pool(name="p_pt", bufs=8))
        p_pvs = top.enter_context(tc.tile_pool(name="p_pvs", bufs=4))
        p_y = top.enter_context(tc.tile_pool(name="p_y", bufs=2))
        ps_main = top.enter_context(
            tc.tile_pool(name="ps_main", bufs=2, space="PSUM"))
        ps_pv = top.enter_context(
            tc.tile_pool(name="ps_pv", bufs=2, space="PSUM"))

        # v3 allocated once: the 64 ones-columns are written a single time
        # (emit_v only ever writes cols 0:DH)
        v3_sb = p_v3.tile([128, NT, H, VW], BF16, name="v3s")
        nc.gpsimd.memset(v3_sb[:, :, :, DH:VW], 1.0)

        rep_iter = [None] if use_loop else list(range(repeat))
        loop_cm = tc.For_i(
            0, repeat, 1,
            hint_engines=(mybir.EngineType.PE, mybir.EngineType.DVE,
                          mybir.EngineType.Activation, mybir.EngineType.SP,
                          mybir.EngineType.Pool),
        ) if use_loop else None
        if loop_cm is not None:
            loop_cm.__enter__()
        for _rep in rep_iter:
            if True:
                wq_sb = p_w.tile([128, 8, D], BF16, tag="wq")
                wk_sb = p_w.tile([128, 8, D], BF16, tag="wk")
                wv_sb = p_w.tile([128, 8, D], BF16, tag="wv")
                wo_sb = p_w.tile([128, 8, D], BF16, tag="wo")
                xts = [p_x.tile([128, 8, CHUNK], BF16, tag="xt",
                                name=f"xt{i}") for i in range(N_CHUNK)]

                # --- input DMA stream (SP queue, consumption order) ---
                for k in range(8):
                    nc.sync.dma_start(xts[0][:, k, :], xt_ap[0, :, k, :])
                for w_sb, w_ap in ((wq_sb, wq_ap), (wk_sb, wk_ap)):
                    nc.sync.dma_start(w_sb[:, :, 0:512], w_ap[:, :, 0:512])
                    nc.sync.dma_start(w_sb[:, :, 512:D], w_ap[:, :, 512:D])
                nc.sync.dma_start(wv_sb[:], wv_ap[:])
                nc.sync.dma_start(wo_sb[:], wo_ap[:])
                if N_CHUNK > 1:
                    nc.sync.dma_start(xts[1][:], xt_ap[1])

                carry = []
                for c in range(N_CHUNK):
                    xt_sb = xts[c]
                    qt_sb = p_qkv.tile([128, 8, CHUNK], BF16, tag="qt")
                    kt_sb = p_qkv.tile([128, 8, CHUNK], BF16, tag="kt")

                    def emit_qk(w_sb, bias_sb, dst, m, n, xt_sb=xt_sb):
                        acc = ps_main.tile([128, 512], F32, tag="acc")
                        for k in range(8):
                            nc.tensor.matmul(
                                acc[:],
                                w_sb[:, k, m * 128:(m + 1) * 128],
                                xt_sb[:, k, n * 512:(n + 1) * 512],
                                start=(k == 0), stop=(k == 7),
                            )
                        nc.scalar.activation(
                            dst[:, m, n * 512:(n + 1) * 512],
                            acc[:],
                            mybir.ActivationFunctionType.Identity,
                            bias=bias_sb[:, m:m + 1],
                        )

                    def emit_qk_n1(m, pi):
                        if pi == 0:
                            emit_qk(wq_sb, bq_sb, qt_sb, m, 1)
                        else:
                            emit_qk(wk_sb, bk_sb, kt_sb, m, 1)

                    if "qk" not in phases:
                        nc.vector.memset(qt_sb[:], 0.5)
                        nc.vector.memset(kt_sb[:], 0.5)

                    # at_sb allocated after S1 so the previous chunk's carried
                    # O readers are registered before this buffer is reused;
                    # see below.

                    def emit_v(i, h2, xt_sb=xt_sb):
                        acc = ps_main.tile([128, 512], F32, tag="acc")
                        for k in range(8):
                            nc.tensor.matmul(
                                acc[:],
                                xt_sb[:, k, i * 128:(i + 1) * 128],
                                wv_sb[:, k, h2 * 512:(h2 + 1) * 512],
                                start=(k == 0), stop=(k == 7),
                            )
                        nc.vector.tensor_add(
                            v3_sb[:, i, h2 * 8:(h2 + 1) * 8, 0:DH],
                            acc[:].rearrange("p (h d) -> p h d", h=8),
                            bv_h[:, h2 * 8:(h2 + 1) * 8, :],
                        )

                    y_tiles = {}

                    def emit_o(i, h2, y_tiles=y_tiles, c=c):
                        # at_sb is bound late (after S1) via at_ref
                        at_sb = at_ref[0]
                        if h2 == 0:
                            y_tiles[i] = p_y.tile([128, D], F32, tag="y",
                                                  name=f"yt{i}")
                        y_t = y_tiles[i]
                        acc = ps_main.tile([128, 512], F32, tag="acc")
                        for k in range(8):
                            nc.tensor.matmul(
                                acc[:],
                                at_sb[:, k, i * 128:(i + 1) * 128],
                                wo_sb[:, k, h2 * 512:(h2 + 1) * 512],
                                start=(k == 0), stop=(k == 7),
                            )
                        nc.vector.tensor_add(
                            y_t[:, h2 * 512:(h2 + 1) * 512], acc[:],
                            bo_sb[:, h2 * 512:(h2 + 1) * 512])
                        if h2 == 1:
                            nc.sync.dma_start(
                                y_ap[c * CHUNK + i * 128:
                                     c * CHUNK + (i + 1) * 128, :],
                                y_t[:])

                    # ---- attention group machinery ----
                    sts = {}     # (b, h) -> (st_psum, pt)
                    pts = {}

                    def emit_st_batch(group, qt_sb=qt_sb, kt_sb=kt_sb):
                        # group: list of (b, h); emit quadrant-interleaved
                        # pairs, then mask-add + exp per head-block
                        tiles = {}
                        for (b, h) in group:
                            st_t = ps_main.tile(
                                [128, ST_W], F32, tag="st", bufs=4,
                                name=f"st{b}_{h}")
                            tiles[(b, h)] = st_t
                        for pi in range(0, len(group), 2):
                            pair = group[pi:pi + 2]
                            for (b, h) in pair:
                                t0 = b * W
                                hp = (h % 2) * 64
                                j = h // 2
                                st = tiles[(b, h)]
                                nc.tensor.matmul(
                                    st[:, 0:W],
                                    kt_sb[hp:hp + 64, j, t0:t0 + 128],
                                    qt_sb[hp:hp + 64, j, t0:t0 + W],
                                    start=True, stop=True,
                                    tile_position=(hp, 0))
                            for (b, h) in pair:
                                t0 = b * W
                                hp = (h % 2) * 64
                                j = h // 2
                                st = tiles[(b, h)]
                                nc.tensor.matmul(
                                    st[:, W:ST_W],
                                    kt_sb[hp:hp + 64, j, t0 + 128:t0 + W],
                                    qt_sb[hp:hp + 64, j, t0 + 128:t0 + W],
                                    start=True, stop=True,
                                    tile_position=(hp, 0),
                                    skip_group_check=True)
                        for (b, h) in group:
                            st = tiles[(b, h)]
                            pt = p_pt.tile([128, ST_W], BF16, tag="pt")
                            nc.scalar.activation(
                                pt[:], st[:],
                                mybir.ActivationFunctionType.Exp, scale=SCALE)
                            nc.gpsimd.tensor_mul(pt[:], pt[:], mf_sb[:])
                            pts[(b, h)] = pt

                    def emit_pv_batch(group, at_getter=lambda: at_ref[0]):
                        at_sb = at_getter()
                        for pi in range(0, len(group), 2):
                            pair = group[pi:pi + 2]
                            pv2 = ps_pv.tile([128, 2 * W], F32, tag="pv2")
                            for ci, (b, h) in enumerate(pair):
                                pt = pts.pop((b, h))
                                c0 = ci * W
                                nc.tensor.matmul(
                                    pv2[:, c0:c0 + W],
                                    v3_sb[:, 2 * b, h, :], pt[:, 0:W],
                                    start=True, stop=False,
                                    skip_group_check=True)
                                nc.tensor.matmul(
                                    pv2[:, c0 + 128:c0 + W],
                                    v3_sb[:, 2 * b + 1, h, :], pt[:, W:ST_W],
                                    start=False, stop=True,
                                    skip_group_check=True)
                            for ci, (b, h) in enumerate(pair):
                                c0 = ci * W
                                t0 = b * W
                                hp = (h % 2) * 64
                                j = h // 2
                                pvf = p_pvs.tile([128, W], F32, tag="pvf",
                                                 bufs=4, name=f"pvf{ci}")
                                if ci == 0:
                                    nc.scalar.copy(
                                        pvf[:], pv2[:, c0:c0 + W])
                                else:
                                    nc.vector.tensor_copy(
                                        pvf[:], pv2[:, c0:c0 + W])
                                rec = p_pvs.tile([64, W], F32, tag="rec",
                                                 bufs=4)
                                nc.vector.reciprocal(
                                    rec[:], pvf[64:128, :])
                                nc.gpsimd.tensor_mul(
                                    at_sb[hp:hp + 64, j, t0:t0 + W],
                                    pvf[0:64, :], rec[:])

                    do_attn = "attn" in phases
                    do_v = "v" in phases
                    do_o = "o" in phases
                    do_qk = "qk" in phases

                    # S1: QK n=0 accs interleaved with carried O accs and the
                    # V accs for blocks 0-1.
                    s1_fill = list(carry)
                    carry = []
                    if do_v:
                        s1_fill += [(emit_v, i, h2)
                                    for i in (0, 1, 2, 3) for h2 in (0, 1)]
                    if do_qk:
                        for m in range(8):
                            emit_qk(wq_sb, bq_sb, qt_sb, m, 0)
                            if s1_fill:
                                fn, a1, a2 = s1_fill.pop(0)
                                fn(a1, a2)
                            emit_qk(wk_sb, bk_sb, kt_sb, m, 0)
                            if s1_fill:
                                fn, a1, a2 = s1_fill.pop(0)
                                fn(a1, a2)
                    for fn, a1, a2 in s1_fill:
                        fn(a1, a2)

                    # bind at AFTER carried O emits are registered
                    at_sb = p_at.tile([128, 8, CHUNK], BF16, tag="at")
                    at_ref = [at_sb]
                    if "v" not in phases:
                        nc.vector.memset(v3_sb[:, :, :, 0:DH], 0.01)
                    if "attn" not in phases:
                        nc.vector.memset(at_sb[:], 0.01)

                    # filler queue, deadline-ordered: early m-tiles of QK n=1
                    # and V i4/i5 first (blocks 2-3 need them from group 8 on)
                    fillq = []
                    if do_qk:
                        qk1 = [(emit_qk_n1, m, pi)
                               for m in range(8) for pi in (0, 1)]
                    else:
                        qk1 = []
                    if do_v:
                        vf = [(emit_v, i, h2)
                              for i in (4, 5, 6, 7) for h2 in (0, 1)]
                    else:
                        vf = []
                    # deadline order (slot p+1 consumes pos p; st(g) needs
                    # its m-tiles by pos 2g-2, pv(g) needs v3 by pos 2g):
                    # qk-n1 m0,m1 | v i4 | m2 | v i5 | m3..m7 | v i6,i7
                    fillq = (qk1[0:4] + vf[0:2] + qk1[4:6] + vf[2:4]
                             + qk1[6:16] + vf[4:8])

                    ready_o = []

                    def filler():
                        if fillq:
                            fn, a1, a2 = fillq.pop(0)
                            fn(a1, a2)
                        elif ready_o and do_o:
                            emit_o(*ready_o.pop(0))

                    if do_attn:
                        hbs = [(b, h) for b in range(NB) for h in range(H)]
                        groups = [hbs[i:i + GSZ]
                                  for i in range(0, len(hbs), GSZ)]
                        prev = None
                        for gi, group in enumerate(groups):
                            emit_st_batch(group)
                            filler()
                            if prev is not None:
                                emit_pv_batch(prev)
                                b_done = prev[-1][0]
                                if prev[-1][1] == H - 1 and do_o:
                                    ready_o.extend(
                                        [(2 * b_done, 0), (2 * b_done, 1),
                                         (2 * b_done + 1, 0),
                                         (2 * b_done + 1, 1)])
                            filler()
                            prev = group
                        emit_pv_batch(prev)
                        if prev[-1][1] == H - 1 and do_o:
                            b_done = prev[-1][0]
                            ready_o.extend(
                                [(2 * b_done, 0), (2 * b_done, 1),
                                 (2 * b_done + 1, 0), (2 * b_done + 1, 1)])
                        for fn, a1, a2 in fillq:
                            fn(a1, a2)
                        fillq = []
                        n_carry = 8 if (c + 1 < N_CHUNK and do_qk
                                        and do_o) else 0
                        if n_carry:
                            carry = [(emit_o, i, h2)
                                     for i, h2 in ready_o[-n_carry:]]
                            ready_o = ready_o[:-n_carry]
                        for i, h2 in ready_o:
                            emit_o(i, h2)
                    else:
                        for fn, a1, a2 in fillq:
                            fn(a1, a2)
                        if do_o:
                            for i in range(NT):
                                for h2 in (0, 1):
                                    emit_o(i, h2)
        if loop_cm is not None:
            loop_cm.__exit__(None, None, None)
    return nc


# ---------------------------------------------------------------------------
_CACHE: dict = {}


def _host_prep(x, Wq, bq, Wk, bk, Wv, bv, Wo, bo):
    import ml_dtypes
    BF = ml_dtypes.bfloat16
    x = np.asarray(x, np.float32)
    x_flat = np.ascontiguousarray(x.reshape(B * S, D)).astype(BF)
    mf = np.ones((128, ST_W), np.float32)
    for p in range(128):
        mf[p, :p] = 0.0
        mf[p, W:W + p] = 0.0
    mf = mf.astype(BF)

    def wfmt(Wm):
        # [128, 8, D]: wfmt[p, k, c] = W[k*128 + p, c]
        return np.ascontiguousarray(
            np.asarray(Wm, np.float32).reshape(8, 128, D)
            .transpose(1, 0, 2).astype(BF))

    def xfmt(shard_x):
        # shard_x [T_CORE, D] -> [N_CHUNK, 128, 8, CHUNK]
        xt = shard_x.T  # [D, T_CORE]
        return np.ascontiguousarray(
            xt.reshape(8, 128, N_CHUNK, CHUNK).transpose(2, 1, 0, 3))

    shard = {
        "xt": np.stack([
            xfmt(x_flat[cix * T_CORE:(cix + 1) * T_CORE])
            for cix in range(N_CORES)
        ]),
    }
    repl = {
        "wq": wfmt(Wq),
        "wk": wfmt(Wk),
        "wv": wfmt(Wv),
        "wo": wfmt(Wo),
        "bqr": np.ascontiguousarray(np.asarray(bq, np.float32).reshape(8, 128).T),
        "bkr": np.ascontiguousarray(np.asarray(bk, np.float32).reshape(8, 128).T),
        "bvb": np.ascontiguousarray(np.tile(np.asarray(bv, np.float32), (128, 1))),
        "bob": np.ascontiguousarray(np.tile(np.asarray(bo, np.float32), (128, 1))),
        "mfull": mf,
    }
    return shard, repl


def _make_runner(repeat: int, use_loop: bool = False,
                 phases=("qk", "v", "attn", "o")):
    """Build program + cached jitted executable. Returns (run, n_outs info)."""
    import jax
    from jax.sharding import Mesh, PartitionSpec
    from jax.experimental.shard_map import shard_map
    from concourse import bass2jax
    from concourse.bass2jax import _bass_exec_p, install_neuronx_cc_hook

    install_neuronx_cc_hook()
    nc = build_program(repeat, use_loop, phases)
    partition_name = (
        nc.partition_id_tensor.name if nc.partition_id_tensor else None
    )
    in_names, out_names, out_avals = [], [], []
    import jax.core
    for alloc in nc.m.functions[0].allocations:
        if not isinstance(alloc, mybir.MemoryLocationSet):
            continue
        name = alloc.memorylocations[0].name
        if alloc.kind == "ExternalInput":
            if name != partition_name:
                in_names.append(name)
        elif alloc.kind == "ExternalOutput":
            out_names.append(name)
            out_avals.append(jax.core.ShapedArray(
                tuple(alloc.tensor_shape), mybir.dt.np(alloc.dtype)))
    all_in_names = list(in_names) + list(out_names)
    if partition_name is not None:
        all_in_names.append(partition_name)

    def _body(*args):
        operands = list(args)
        if partition_name is not None:
            operands.append(bass2jax.partition_id_tensor())
        return tuple(_bass_exec_p.bind(
            *operands,
            out_avals=tuple(out_avals),
            in_names=tuple(all_in_names),
            out_names=tuple(out_names),
            lowering_input_output_aliases=(),
            sim_require_finite=True,
            sim_require_nnan=True,
            nc=nc,
        ))

    import jax as _jax
    devices = _jax.devices()[:N_CORES]
    mesh = Mesh(np.asarray(devices), ("core",))
    SHARDED_INS = {"xt"}
    in_specs = tuple(
        PartitionSpec("core") if n in SHARDED_INS else PartitionSpec()
        for n in in_names
    ) + (PartitionSpec("core"),) * len(out_names)
    out_specs = (PartitionSpec("core"),) * len(out_names)
    sharded = _jax.jit(
        shard_map(_body, mesh=mesh, in_specs=in_specs,
                  out_specs=out_specs, check_rep=False),
        keep_unused=True,
    )

    from jax.sharding import NamedSharding
    sh_core = NamedSharding(mesh, PartitionSpec("core"))
    sh_repl = NamedSharding(mesh, PartitionSpec())

    def _args(shard_arrs: dict, repl_arrs: dict):
        args, shs = [], []
        for n in in_names:
            if n in SHARDED_INS:
                a = shard_arrs[n]
                args.append(a.reshape(a.shape[0] * a.shape[1], *a.shape[2:]))
                shs.append(sh_core)
            else:
                args.append(repl_arrs[n])
                shs.append(sh_repl)
        for av in out_avals:
            args.append(np.zeros((N_CORES * av.shape[0], *av.shape[1:]),
                                 av.dtype))
            shs.append(sh_core)
        return args, shs

    class Runner:
        def stage(self, shard_arrs, repl_arrs):
            args, shs = _args(shard_arrs, repl_arrs)
            dargs = [_jax.device_put(a, s) for a, s in zip(args, shs)]
            _jax.block_until_ready(dargs)
            return dargs

        def exec_staged(self, dargs):
            outs = sharded(*dargs)
            _jax.block_until_ready(outs)
            return outs

        def run(self, shard_arrs, repl_arrs):
            args, _ = _args(shard_arrs, repl_arrs)
            outs = sharded(*args)
            _jax.block_until_ready(outs)
            return {
                name: np.asarray(outs[i]).reshape(N_CORES, *out_avals[i].shape)
                for i, name in enumerate(out_names)
            }

    return Runner()


def get_runner(repeat: int = 1, use_loop: bool = False,
               phases=("qk", "v", "attn", "o")):
    key = ("runner", repeat, use_loop, tuple(phases))
    if key not in _CACHE:
        _CACHE[key] = _make_runner(repeat, use_loop, phases)
    return _CACHE[key]


def kernel(**inputs) -> np.ndarray:
    runner = get_runner(repeat=1)
    shard, repl = _host_prep(**inputs)
    out = runner.run(shard, repl)
    y = out["y"].reshape(B * S, D)
    return y.reshape(B, S, D).astype(np.float32)
